# revision 19
# baseline (speedup 1.0000x reference)
"""Causal self-attention with RoPE on 8 trn2 NeuronCores.

Sharding: tensor-parallel over heads (Megatron style). 16 heads, 8 cores
-> 2 heads per core. Each core computes q/k/v for its 2 heads, causal
attention, and a partial output projection against its w_o column slice.
Host sums the 8 partial outputs (the Megatron all-reduce, done at gather).

Device-side design (bf16 compute, fp32 PSUM accumulation throughout):
 - xT [C, B*T] bf16: x pre-transposed on host so the QKV contraction dim
   (c) is on SBUF partitions; no on-device transpose of x.
 - w_qk packed per head into even/odd-dim column blocks [QE|QO|KE|KO];
   QKV matmuls produce q/k directly in [d, t] layout, head-stacked so
   RoPE runs full-128-partition DVE tensor_tensor ops (6 per tile).
   PSUM is freed via one wide ScalarE copy per tile; rope reads SBUF.
 - v in natural [t, d] layout (same x tiles, shared j/c loop), stored
   with a ones column per head: AV then yields y[tq, 0:128] AND the
   softmax denominator y[:, 128] from a single accumulated matmul.
 - Post-rope q/k repacked (SBUF->SBUF DMA) into per-head [d, t] tensors
   so scores are single K=128 matmuls: scoresT[ts, tq] = khat.T @ qhat.
 - Softmax: no max-subtraction (scores ~N(0,1)); exp on ScalarE with the
   1/sqrt(D) scale folded in, emitted over paired 1024-wide tq blocks to
   amortize per-instruction overhead; causal masking only on
   diagonal-touching tiles via 0/1 bf16 mask multiply.
 - Normalize with reciprocal + per-partition tensor_scalar, PE-transpose
   y -> yT.
 - Output projection is cut into per-token-tile jobs and interleaved
   into the exp-bound attention instruction stream and the next batch's
   QK phase, so the PE never sits idle waiting on ScalarE exp. Partial
   outputs are stored bf16 (halves store bandwidth), one wide DMA per
   128-token row block.
 - Resident weights stream in per-c-tile chunks so the first QKV matmul
   issues ~1us after kernel start instead of waiting for the full load.

PSUM budget (8 banks of 2KB): tag "big" 3 bufs x [128,1024] f32
(6 banks) carries ps_q/ps_k/packed-ps_v in the QK phase and
sc/y_ps-packs/transpose targets in attention; tag "po" 2 bufs x
[128,512] f32 (2 banks) carries the interleaved output-projection
accumulators.

Compile recipe (this container): bacc.Bacc("TRN2") + TileContext +
nc.finalize() before run_bass_kernel_spmd (bacc legalizes multi-wait
instructions; raw bass.Bass fails walrus codegen here).
"""

import math

import numpy as np

B, T, C, H = 2, 2048, 2048, 16
D = C // H  # 128
NCORES = 8
HPC = H // NCORES  # heads per core = 2
N = B * T  # 4096 token rows
TT = T // 128  # 16 t-tiles per batch
NB = T // 512  # 4 n/tq blocks of 512 per batch
CT = C // 128  # 16 contraction tiles

_COMPILED = None


def _build():
    import concourse.bacc as bacc
    import concourse.mybir as mybir
    import concourse.tile as tile
    from concourse.masks import make_identity

    f32 = mybir.dt.float32
    bf16 = mybir.dt.bfloat16

    nc = bacc.Bacc("TRN2", target_bir_lowering=False, debug=False)
    xT = nc.declare_dram_parameter("xT", [C, N], bf16, isOutput=False)
    w_qk = nc.declare_dram_parameter("w_qk", [C, 4 * D], bf16, isOutput=False)
    w_v = nc.declare_dram_parameter("w_v", [C, HPC * D], bf16, isOutput=False)
    w_o = nc.declare_dram_parameter("w_o", [HPC * D, C], bf16, isOutput=False)
    cos2 = nc.declare_dram_parameter("cos2", [D, N], bf16, isOutput=False)
    sin2 = nc.declare_dram_parameter("sin2", [D, N], bf16, isOutput=False)
    # single [128,128] causal triangle: only the one partial 128-col strip
    # of each diagonal score tile needs masking (strips fully above the
    # diagonal are never read by AV; strips below multiply by 1).
    masks = nc.declare_dram_parameter("masks", [128, 128], bf16, isOutput=False)
    out_p = nc.declare_dram_parameter("out_p", [N, C], bf16, isOutput=True)

    SCALE = 1.0 / math.sqrt(D)
    VW = HPC * D + 2 * HPC  # 260: per t-tile v storage [v_h0|1|pad|v_h1|1|pad]

    with tile.TileContext(nc) as tc:
        with (
            tc.tile_pool(name="wpool", bufs=1) as wpool,
            tc.tile_pool(name="xpool", bufs=12) as xpool,
            tc.tile_pool(name="eo", bufs=4) as eopool,
            tc.tile_pool(name="ropetmp", bufs=4) as tmppool,
            tc.tile_pool(name="vsb", bufs=1) as vpool,
            tc.tile_pool(name="expp", bufs=18) as exppool,
            tc.tile_pool(name="ysb", bufs=12) as ypool,
            tc.tile_pool(name="rsb", bufs=8) as rpool,
            tc.tile_pool(name="yos", bufs=2) as yopool,
            tc.tile_pool(name="pbig", bufs=3, space="PSUM") as pbig,
            tc.tile_pool(name="pout", bufs=2, space="PSUM") as pop,
        ):
            # ---- resident weights / constants, chunked so QK starts fast;
            # wqk on sync, wv on the scalar queue (parallel DGE queues) ----
            wqk_sb = wpool.tile([128, CT * 512], bf16, tag="wqk")
            wv_sb = wpool.tile([128, CT * 256], bf16, tag="wv")
            xwarm = []
            for c in range(4):
                xt = xpool.tile([128, 512], bf16, tag="xt", name="xw")
                nc.sync.dma_start(out=xt[:, :], in_=xT[c * 128 : (c + 1) * 128, 0:512])
                xwarm.append(xt)
                nc.sync.dma_start(
                    out=wqk_sb[:, c * 512 : (c + 1) * 512],
                    in_=w_qk[c * 128 : (c + 1) * 128, :],
                )
                nc.scalar.dma_start(
                    out=wv_sb[:, c * 256 : (c + 1) * 256],
                    in_=w_v[c * 128 : (c + 1) * 128, :],
                )
            for c in range(4, CT):
                nc.sync.dma_start(
                    out=wqk_sb[:, c * 512 : (c + 1) * 512],
                    in_=w_qk[c * 128 : (c + 1) * 128, :],
                )
                nc.scalar.dma_start(
                    out=wv_sb[:, c * 256 : (c + 1) * 256],
                    in_=w_v[c * 128 : (c + 1) * 128, :],
                )
            cos_sb = wpool.tile([128, N], bf16, tag="cos")
            nc.sync.dma_start(out=cos_sb[:, :], in_=cos2[:, :])
            sin_sb = wpool.tile([128, N], bf16, tag="sin")
            nc.sync.dma_start(out=sin_sb[:, :], in_=sin2[:, :])
            mask_sb = wpool.tile([128, 128], bf16, tag="mask")
            nc.sync.dma_start(out=mask_sb[:, :], in_=masks[:, :])
            wo_sb = wpool.tile([128, HPC * C], bf16, tag="wo")
            for h in range(HPC):
                nc.sync.dma_start(
                    out=wo_sb[:, h * C : (h + 1) * C],
                    in_=w_o[h * 128 : (h + 1) * 128, :],
                )
            ident = wpool.tile([128, 128], bf16, tag="ident")
            make_identity(nc, ident[:, :])

            v_sb = vpool.tile([128, TT * VW], bf16, tag="vsb")
            for tt in range(TT):
                for h in range(HPC):
                    col = tt * VW + h * 130 + 128
                    nc.vector.memset(v_sb[:, col : col + 1], 1.0)

            # ---- interleaved work queues: block finishers (transposes +
            # yT copies, emitted late so PE never waits on the normalize)
            # and output-projection jobs ----
            finq = []
            wo_jobs = []

            def drain(k, eng="scalar"):
                for _ in range(k):
                    if finq:
                        finq.pop(0)()
                    elif wo_jobs:
                        wo_jobs.pop(0)(eng)
                    else:
                        return

            def push_wo(n0, yT, tts):
                for tt in tts:
                    def job(eng, tt=tt, n0=n0, yT=yT):
                        tsl = slice(tt * 128, (tt + 1) * 128)
                        yo = yopool.tile([128, C], bf16, tag="yo", name="yo")
                        for ob in range(4):
                            o_ps = pop.tile([128, 512], f32, tag="po", name="o_ps")
                            for h in range(HPC):
                                nc.tensor.matmul(
                                    o_ps[:, :],
                                    yT[h][:, tsl],
                                    wo_sb[:, h * C + ob * 512 : h * C + (ob + 1) * 512],
                                    start=(h == 0),
                                    stop=(h == HPC - 1),
                                )
                            osl = yo[:, ob * 512 : (ob + 1) * 512]
                            # PSUM->SBUF bounce: split across engines so no
                            # single engine eats the whole drain cost.
                            if eng == "scalar":
                                nc.scalar.copy(osl, o_ps[:, :])
                            elif eng == "mixed":
                                if ob % 2 == 0:
                                    nc.scalar.copy(osl, o_ps[:, :])
                                else:
                                    nc.vector.tensor_copy(osl, o_ps[:, :])
                            else:
                                nc.vector.tensor_copy(osl, o_ps[:, :])
                        nc.sync.dma_start(
                            out=out_p[n0 + tt * 128 : n0 + (tt + 1) * 128, :],
                            in_=yo[:, :],
                        )
                    wo_jobs.append(job)

            for b in range(B):
                n0 = b * T
                deferred_rope = []

                # ---- phase QK: q,k projection in [d, t] layout + RoPE ----
                qe2 = eopool.tile([128, T], bf16, tag="eo", name="qe2")
                qo2 = eopool.tile([128, T], bf16, tag="eo", name="qo2")
                ke2 = eopool.tile([128, T], bf16, tag="eo", name="ke2")
                ko2 = eopool.tile([128, T], bf16, tag="eo", name="ko2")
                rot = [(qe2, qo2), (ke2, ko2)]
                qhat = [eopool.tile([128, T], bf16, tag="qh", name=f"qhat{_h}") for _h in range(HPC)]
                khat = [eopool.tile([128, T], bf16, tag="qh", name=f"khat{_h}") for _h in range(HPC)]
                for j in range(NB):
                    js = slice(j * 512, (j + 1) * 512)
                    ps_q = pbig.tile([128, 1024], f32, tag="big", name="ps_q")
                    ps_k = pbig.tile([128, 1024], f32, tag="big", name="ps_k")
                    # 4 v accumulation groups, each owning a PSUM bank:
                    # tl 0/1 -> po pool slots (idle during the c-loop),
                    # tl 2/3 -> bank 0 / bank 1 of one big slot.
                    ps_v = pbig.tile([128, 1024], f32, tag="big", name="ps_v")
                    ps_va = pop.tile([128, 256], f32, tag="po", name="ps_va")
                    ps_vb = pop.tile([128, 256], f32, tag="po", name="ps_vb")
                    vdst = [
                        ps_va[:, 0:256],
                        ps_vb[:, 0:256],
                        ps_v[:, 0:256],
                        ps_v[:, 512:768],
                    ]
                    for c in range(CT):
                        if b == 0 and j == 0 and c < 4:
                            xt = xwarm[c]
                        else:
                            xt = xpool.tile([128, 512], bf16, tag="xt")
                            nc.gpsimd.dma_start(
                                out=xt[:, :],
                                in_=xT[c * 128 : (c + 1) * 128, n0 + j * 512 : n0 + (j + 1) * 512],
                            )
                        for part in range(4):  # QE, QO, KE, KO
                            dst = (ps_q, ps_q, ps_k, ps_k)[part]
                            off = (0, 512, 0, 512)[part]
                            wsl = wqk_sb[:, c * 512 + part * 128 : c * 512 + (part + 1) * 128]
                            nc.tensor.matmul(
                                dst[:, off : off + 512],
                                wsl,
                                xt[:, :],
                                start=(c == 0),
                                stop=(c == CT - 1),
                            )
                        for tl in range(4):
                            nc.tensor.matmul(
                                vdst[tl],
                                xt[:, tl * 128 : (tl + 1) * 128],
                                wv_sb[:, c * 256 : (c + 1) * 256],
                                start=(c == 0),
                                stop=(c == CT - 1),
                            )
                    # v copies first (frees the v psum slots for j+1)
                    for tl in range(4):
                        tt = j * 4 + tl
                        base = tt * VW
                        for h in range(HPC):
                            nc.vector.tensor_copy(
                                v_sb[:, base + h * 130 : base + h * 130 + 128],
                                vdst[tl][:, h * 128 : (h + 1) * 128],
                            )
                    # ACT copy frees psum banks fast, quantizing to bf16 so
                    # rope runs at DVE 2x; rope of the LAST j block is
                    # deferred off the attention-start critical path.
                    pcs = []
                    for qk in range(2):  # 0 = q, 1 = k
                        pc = tmppool.tile([128, 1024], bf16, tag="rt", name=f"pc{qk}")
                        nc.scalar.copy(pc[:, :], (ps_q, ps_k)[qk][:, :])
                        pcs.append(pc)

                    def emit_rope(j=j, js=js, pcs=pcs):
                        ce = cos_sb[:, n0 + j * 512 : n0 + (j + 1) * 512]
                        se = sin_sb[:, n0 + j * 512 : n0 + (j + 1) * 512]
                        for qk in range(2):
                            E_sb, O_sb = pcs[qk][:, 0:512], pcs[qk][:, 512:1024]
                            dst_e, dst_o = rot[qk]
                            t1 = tmppool.tile([128, 512], bf16, tag="rt2")
                            t2 = tmppool.tile([128, 512], bf16, tag="rt2")
                            nc.vector.tensor_mul(t1[:, :], E_sb, ce)
                            nc.vector.tensor_mul(t2[:, :], O_sb, se)
                            nc.vector.tensor_sub(dst_e[:, js], t1[:, :], t2[:, :])
                            t3 = tmppool.tile([128, 512], bf16, tag="rt2")
                            t4 = tmppool.tile([128, 512], bf16, tag="rt2")
                            nc.vector.tensor_mul(t3[:, :], E_sb, se)
                            nc.vector.tensor_mul(t4[:, :], O_sb, ce)
                            nc.vector.tensor_add(dst_o[:, js], t3[:, :], t4[:, :])
                        for h in range(HPC):
                            hb = 64 * h
                            nc.sync.dma_start(out=qhat[h][0:64, js], in_=qe2[hb : hb + 64, js])
                            nc.sync.dma_start(out=qhat[h][64:128, js], in_=qo2[hb : hb + 64, js])
                            nc.sync.dma_start(out=khat[h][0:64, js], in_=ke2[hb : hb + 64, js])
                            nc.sync.dma_start(out=khat[h][64:128, js], in_=ko2[hb : hb + 64, js])

                    if j < NB - 1:
                        emit_rope()
                        drain(2)
                    else:
                        deferred_rope.append(emit_rope)

                # ---- attention per head: paired tq blocks (jlo, jhi) share
                # one [128,1024] score psum + one wide exp instruction ----
                yT = [eopool.tile([128, T], bf16, tag="yt", name=f"yT{_h}") for _h in range(HPC)]
                for jp in range(NB // 2):
                    jlo, jhi = 2 * jp, 2 * jp + 1
                    for h in range(HPC):
                        if jp == 0 and h == 1 and deferred_rope:
                            deferred_rope.pop(0)()
                        exp_of = {}  # i -> (tile, base col of jlo half or None)
                        for i in range(4 * jhi + 4):
                            isl = slice(i * 128, (i + 1) * 128)
                            combined = i <= 4 * jlo + 3
                            sc = pbig.tile([128, 1024], f32, tag="big", name="sc")
                            ex = exppool.tile([128, 1024], bf16, tag="ex")
                            if combined:
                                nc.tensor.matmul(
                                    sc[:, 0:512], khat[h][:, isl],
                                    qhat[h][:, jlo * 512 : (jlo + 1) * 512],
                                    start=True, stop=True,
                                )
                                nc.tensor.matmul(
                                    sc[:, 512:1024], khat[h][:, isl],
                                    qhat[h][:, jhi * 512 : (jhi + 1) * 512],
                                    start=True, stop=True,
                                )
                                nc.scalar.activation(
                                    ex[:, :], sc[:, :],
                                    mybir.ActivationFunctionType.Exp, scale=SCALE,
                                )
                                p = i - 4 * jlo
                                if p >= 0:
                                    nc.gpsimd.tensor_mul(
                                        ex[:, p * 128 : (p + 1) * 128],
                                        ex[:, p * 128 : (p + 1) * 128],
                                        mask_sb[:, :],
                                    )
                                exp_of[i] = (ex, 0)
                            else:
                                nc.tensor.matmul(
                                    sc[:, 0:512], khat[h][:, isl],
                                    qhat[h][:, jhi * 512 : (jhi + 1) * 512],
                                    start=True, stop=True,
                                )
                                nc.scalar.activation(
                                    ex[:, 0:512], sc[:, 0:512],
                                    mybir.ActivationFunctionType.Exp, scale=SCALE,
                                )
                                p = i - 4 * jhi
                                if p >= 0:
                                    nc.gpsimd.tensor_mul(
                                        ex[:, p * 128 : (p + 1) * 128],
                                        ex[:, p * 128 : (p + 1) * 128],
                                        mask_sb[:, :],
                                    )
                                exp_of[i] = (ex, None)
                            if i % 2 == 1:
                                drain(1, "mixed")

                        for j in (jlo, jhi):
                            half = 0 if j == jlo else 512
                            # 2 taus per packed psum tile (2x129 f32 regions)
                            ypk = [pbig.tile([128, 1024], f32, tag="big", name=f"ypk{_p}") for _p in range(2)]
                            for tau in range(4):
                                g = 4 * j + tau
                                dst = ypk[tau // 2]
                                yof = (tau % 2) * 512
                                for i in range(g + 1):
                                    ex, base = exp_of[i]
                                    col = (half if base == 0 else 0) + tau * 128
                                    nc.tensor.matmul(
                                        dst[:, yof : yof + 129],
                                        ex[:, col : col + 128],
                                        v_sb[:, i * VW + h * 130 : i * VW + h * 130 + 129],
                                        start=(i == 0),
                                        stop=(i == g),
                                    )
                                if tau % 2 == 1:
                                    drain(1, "mixed")
                            y_sb4 = []
                            for tau in range(4):
                                srcp = ypk[tau // 2]
                                yof = (tau % 2) * 512
                                r = rpool.tile([128, 1], f32, tag="r")
                                nc.vector.reciprocal(r[:, :], srcp[:, yof + 128 : yof + 129])
                                y_sb = ypool.tile([128, 128], bf16, tag="y")
                                nc.vector.tensor_scalar_mul(
                                    y_sb[:, :], srcp[:, yof : yof + 128], r[:, 0:1]
                                )
                                y_sb4.append(y_sb)

                            def finisher(j=j, h=h, y_sb4=y_sb4, n0=n0, yT=yT):
                                for tau in range(4):
                                    g = 4 * j + tau
                                    yt_ps = pop.tile([128, 128], bf16, tag="po", name="yt_ps")
                                    nc.tensor.transpose(yt_ps[:, :], y_sb4[tau][:, :], ident[:, :])
                                    nc.vector.tensor_copy(
                                        yT[h][:, g * 128 : (g + 1) * 128], yt_ps[:, :]
                                    )
                                if h == HPC - 1:
                                    push_wo(n0, yT, range(4 * j, 4 * j + 4))
                            finq.append(finisher)
                while finq:
                    finq.pop(0)()
            while finq or wo_jobs:
                drain(1, "mixed")

    nc.finalize()
    return nc


def _prep_inputs(x, w_qkv, w_o, rope_cos, rope_sin):
    import ml_dtypes

    bf = ml_dtypes.bfloat16
    xTh = np.ascontiguousarray(x.reshape(N, C).T).astype(bf)
    cosT = np.ascontiguousarray(rope_cos.T)  # [64, T]
    sinT = np.ascontiguousarray(rope_sin.T)
    cos2 = np.tile(np.concatenate([cosT, cosT], 0), (1, B)).astype(bf)
    sin2 = np.tile(np.concatenate([sinT, sinT], 0), (1, B)).astype(bf)

    r = np.arange(128)[:, None]
    c = np.arange(128)[None, :]
    mk = ((c - r) >= 0).astype(np.float32).astype(bf)

    ev = np.arange(0, D, 2)
    od = np.arange(1, D, 2)
    in_maps = []
    for m in range(NCORES):
        h0, h1 = 2 * m, 2 * m + 1
        # blocks QE|QO|KE|KO; within each, cols = [head0 dims | head1 dims]
        QE = np.concatenate([w_qkv[h0 * D + ev, :], w_qkv[h1 * D + ev, :]], 0).T
        QO = np.concatenate([w_qkv[h0 * D + od, :], w_qkv[h1 * D + od, :]], 0).T
        KE = np.concatenate([w_qkv[C + h0 * D + ev, :], w_qkv[C + h1 * D + ev, :]], 0).T
        KO = np.concatenate([w_qkv[C + h0 * D + od, :], w_qkv[C + h1 * D + od, :]], 0).T
        wqk_m = np.ascontiguousarray(np.concatenate([QE, QO, KE, KO], 1)).astype(bf)
        wv_m = np.ascontiguousarray(
            w_qkv[2 * C + 2 * m * D : 2 * C + (2 * m + 2) * D, :].T
        ).astype(bf)
        wo_m = np.ascontiguousarray(w_o[:, 2 * m * D : (2 * m + 2) * D].T).astype(bf)
        in_maps.append(
            {
                "xT": xTh,
                "w_qk": wqk_m,
                "w_v": wv_m,
                "w_o": wo_m,
                "cos2": cos2,
                "sin2": sin2,
                "masks": np.ascontiguousarray(mk),
            }
        )
    return in_maps


def kernel(x, w_qkv, w_o, rope_cos, rope_sin, _trace=False):
    global _COMPILED
    x = np.asarray(x, dtype=np.float32)
    w_qkv = np.asarray(w_qkv, dtype=np.float32)
    w_o = np.asarray(w_o, dtype=np.float32)
    rope_cos = np.asarray(rope_cos, dtype=np.float32)
    rope_sin = np.asarray(rope_sin, dtype=np.float32)

    from concourse.bass_utils import run_bass_kernel_spmd

    if _COMPILED is None:
        _COMPILED = _build()
    nc = _COMPILED
    in_maps = _prep_inputs(x, w_qkv, w_o, rope_cos, rope_sin)
    res = run_bass_kernel_spmd(
        nc, in_maps, core_ids=list(range(NCORES)), trace=_trace
    )
    out = np.zeros((N, C), dtype=np.float32)
    for m in range(NCORES):
        out += np.asarray(res.results[m]["out_p"], dtype=np.float32)
    kernel._last_results = res
    return out.reshape(B, T, C)


# revision 20
# speedup vs baseline: 1.0608x; 1.0608x over previous
"""Causal self-attention with RoPE on 8 trn2 NeuronCores.

Sharding: tensor-parallel over heads (Megatron style). 16 heads, 8 cores
-> 2 heads per core. Each core computes q/k/v for its 2 heads, causal
attention, and a partial output projection against its w_o column slice.
Host sums the 8 partial outputs (the Megatron all-reduce, done at gather).

Device-side design (bf16 compute, fp32 PSUM accumulation throughout):
 - xT [C, B*T] bf16: x pre-transposed on host so the QKV contraction dim
   (c) is on SBUF partitions; no on-device transpose of x.
 - w_qk packed per head into even/odd-dim column blocks [QE|QO|KE|KO];
   QKV matmuls produce q/k directly in [d, t] layout, head-stacked so
   RoPE runs full-128-partition DVE tensor_tensor ops (6 per tile).
   PSUM is freed via one wide ScalarE copy per tile; rope reads SBUF.
 - v in natural [t, d] layout (same x tiles, shared j/c loop), stored
   with a ones column per head: AV then yields y[tq, 0:128] AND the
   softmax denominator y[:, 128] from a single accumulated matmul.
 - Post-rope q/k repacked (SBUF->SBUF DMA) into per-head [d, t] tensors
   so scores are single K=128 matmuls: scoresT[ts, tq] = khat.T @ qhat.
 - Softmax: no max-subtraction (scores ~N(0,1)); exp on ScalarE with the
   1/sqrt(D) scale folded in, emitted over paired 1024-wide tq blocks to
   amortize per-instruction overhead; causal masking only on
   diagonal-touching tiles via 0/1 bf16 mask multiply.
 - Normalize with reciprocal + per-partition tensor_scalar, PE-transpose
   y -> yT.
 - Output projection is cut into per-token-tile jobs and interleaved
   into the exp-bound attention instruction stream and the next batch's
   QK phase, so the PE never sits idle waiting on ScalarE exp. Partial
   outputs are stored bf16 (halves store bandwidth), one wide DMA per
   128-token row block.
 - Resident weights stream in per-c-tile chunks so the first QKV matmul
   issues ~1us after kernel start instead of waiting for the full load.

PSUM budget (8 banks of 2KB): tag "big" 3 bufs x [128,1024] f32
(6 banks) carries ps_q/ps_k/packed-ps_v in the QK phase and
sc/y_ps-packs/transpose targets in attention; tag "po" 2 bufs x
[128,512] f32 (2 banks) carries the interleaved output-projection
accumulators.

Compile recipe (this container): bacc.Bacc("TRN2") + TileContext +
nc.finalize() before run_bass_kernel_spmd (bacc legalizes multi-wait
instructions; raw bass.Bass fails walrus codegen here).
"""

import math

import numpy as np

B, T, C, H = 2, 2048, 2048, 16
D = C // H  # 128
NCORES = 8
HPC = H // NCORES  # heads per core = 2
N = B * T  # 4096 token rows
TT = T // 128  # 16 t-tiles per batch
NB = T // 512  # 4 n/tq blocks of 512 per batch
CT = C // 128  # 16 contraction tiles

_COMPILED = None


def _build():
    import concourse.bacc as bacc
    import concourse.mybir as mybir
    import concourse.tile as tile
    from concourse.masks import make_identity

    f32 = mybir.dt.float32
    bf16 = mybir.dt.bfloat16

    nc = bacc.Bacc("TRN2", target_bir_lowering=False, debug=False)
    xT = nc.declare_dram_parameter("xT", [C, N], bf16, isOutput=False)
    w_qk = nc.declare_dram_parameter("w_qk", [C, 4 * D], bf16, isOutput=False)
    w_v = nc.declare_dram_parameter("w_v", [C, HPC * D], bf16, isOutput=False)
    w_o = nc.declare_dram_parameter("w_o", [HPC * D, C], bf16, isOutput=False)
    cos2 = nc.declare_dram_parameter("cos2", [D, N], bf16, isOutput=False)
    sin2 = nc.declare_dram_parameter("sin2", [D, N], bf16, isOutput=False)
    # single [128,128] causal triangle: only the one partial 128-col strip
    # of each diagonal score tile needs masking (strips fully above the
    # diagonal are never read by AV; strips below multiply by 1).
    masks = nc.declare_dram_parameter("masks", [128, 128], bf16, isOutput=False)
    out_p = nc.declare_dram_parameter("out_p", [N, C], bf16, isOutput=True)

    SCALE = 1.0 / math.sqrt(D)
    VW = HPC * D + 2 * HPC  # 260: per t-tile v storage [v_h0|1|pad|v_h1|1|pad]

    with tile.TileContext(nc) as tc:
        with (
            tc.tile_pool(name="wpool", bufs=1) as wpool,
            tc.tile_pool(name="xpool", bufs=12) as xpool,
            tc.tile_pool(name="eo", bufs=4) as eopool,
            tc.tile_pool(name="ropetmp", bufs=4) as tmppool,
            tc.tile_pool(name="vsb", bufs=1) as vpool,
            tc.tile_pool(name="expp", bufs=18) as exppool,
            tc.tile_pool(name="ysb", bufs=12) as ypool,
            tc.tile_pool(name="rsb", bufs=8) as rpool,
            tc.tile_pool(name="yos", bufs=2) as yopool,
            tc.tile_pool(name="pbig", bufs=3, space="PSUM") as pbig,
            tc.tile_pool(name="pout", bufs=2, space="PSUM") as pop,
        ):
            # ---- resident weights / constants, chunked so QK starts fast;
            # wqk on sync, wv on the scalar queue (parallel DGE queues) ----
            wqk_sb = wpool.tile([128, CT * 512], bf16, tag="wqk")
            wv_sb = wpool.tile([128, CT * 256], bf16, tag="wv")
            xwarm = []
            for c in range(4):
                xt = xpool.tile([128, 512], bf16, tag="xt", name="xw")
                nc.sync.dma_start(out=xt[:, :], in_=xT[c * 128 : (c + 1) * 128, 0:512])
                xwarm.append(xt)
                nc.sync.dma_start(
                    out=wqk_sb[:, c * 512 : (c + 1) * 512],
                    in_=w_qk[c * 128 : (c + 1) * 128, :],
                )
                nc.scalar.dma_start(
                    out=wv_sb[:, c * 256 : (c + 1) * 256],
                    in_=w_v[c * 128 : (c + 1) * 128, :],
                )
            for c in range(4, CT):
                nc.sync.dma_start(
                    out=wqk_sb[:, c * 512 : (c + 1) * 512],
                    in_=w_qk[c * 128 : (c + 1) * 128, :],
                )
                nc.scalar.dma_start(
                    out=wv_sb[:, c * 256 : (c + 1) * 256],
                    in_=w_v[c * 128 : (c + 1) * 128, :],
                )
            cos_sb = wpool.tile([128, N], bf16, tag="cos")
            nc.sync.dma_start(out=cos_sb[:, :], in_=cos2[:, :])
            sin_sb = wpool.tile([128, N], bf16, tag="sin")
            nc.sync.dma_start(out=sin_sb[:, :], in_=sin2[:, :])
            mask_sb = wpool.tile([128, 128], bf16, tag="mask")
            nc.sync.dma_start(out=mask_sb[:, :], in_=masks[:, :])
            wo_sb = wpool.tile([128, HPC * C], bf16, tag="wo")
            for h in range(HPC):
                nc.sync.dma_start(
                    out=wo_sb[:, h * C : (h + 1) * C],
                    in_=w_o[h * 128 : (h + 1) * 128, :],
                )
            ident = wpool.tile([128, 128], bf16, tag="ident")
            make_identity(nc, ident[:, :])

            v_sb = vpool.tile([128, TT * VW], bf16, tag="vsb")
            for tt in range(TT):
                for h in range(HPC):
                    col = tt * VW + h * 130 + 128
                    nc.vector.memset(v_sb[:, col : col + 1], 1.0)

            # ---- interleaved work queues: block finishers (transposes +
            # yT copies, emitted late so PE never waits on the normalize)
            # and output-projection jobs ----
            finq = []
            wo_jobs = []

            def drain(k, eng="scalar"):
                for _ in range(k):
                    if finq:
                        finq.pop(0)()
                    elif wo_jobs:
                        wo_jobs.pop(0)(eng)
                    else:
                        return

            def push_wo(n0, yT, tts):
                for tt in tts:
                    def job(eng, tt=tt, n0=n0, yT=yT):
                        tsl = slice(tt * 128, (tt + 1) * 128)
                        yo = yopool.tile([128, C], bf16, tag="yo", name="yo")
                        for ob in range(4):
                            o_ps = pop.tile([128, 512], f32, tag="po", name="o_ps")
                            for h in range(HPC):
                                nc.tensor.matmul(
                                    o_ps[:, :],
                                    yT[h][:, tsl],
                                    wo_sb[:, h * C + ob * 512 : h * C + (ob + 1) * 512],
                                    start=(h == 0),
                                    stop=(h == HPC - 1),
                                )
                            osl = yo[:, ob * 512 : (ob + 1) * 512]
                            # PSUM->SBUF bounce: split across engines so no
                            # single engine eats the whole drain cost.
                            if eng == "scalar":
                                nc.scalar.copy(osl, o_ps[:, :])
                            elif eng == "mixed":
                                if ob % 2 == 0:
                                    nc.scalar.copy(osl, o_ps[:, :])
                                else:
                                    nc.vector.tensor_copy(osl, o_ps[:, :])
                            else:
                                nc.vector.tensor_copy(osl, o_ps[:, :])
                        nc.sync.dma_start(
                            out=out_p[n0 + tt * 128 : n0 + (tt + 1) * 128, :],
                            in_=yo[:, :],
                        )
                    wo_jobs.append(job)

            for b in range(B):
                n0 = b * T
                deferred_rope = []

                # ---- phase QK: q,k projection in [d, t] layout + RoPE ----
                qe2 = eopool.tile([128, T], bf16, tag="eo", name="qe2")
                qo2 = eopool.tile([128, T], bf16, tag="eo", name="qo2")
                ke2 = eopool.tile([128, T], bf16, tag="eo", name="ke2")
                ko2 = eopool.tile([128, T], bf16, tag="eo", name="ko2")
                rot = [(qe2, qo2), (ke2, ko2)]
                qhat = [eopool.tile([128, T], bf16, tag="qh", name=f"qhat{_h}") for _h in range(HPC)]
                khat = [eopool.tile([128, T], bf16, tag="qh", name=f"khat{_h}") for _h in range(HPC)]
                for j in range(NB):
                    js = slice(j * 512, (j + 1) * 512)
                    ps_q = pbig.tile([128, 1024], f32, tag="big", name="ps_q")
                    ps_k = pbig.tile([128, 1024], f32, tag="big", name="ps_k")
                    # 4 v accumulation groups, each owning a PSUM bank:
                    # tl 0/1 -> po pool slots (idle during the c-loop),
                    # tl 2/3 -> bank 0 / bank 1 of one big slot.
                    ps_v = pbig.tile([128, 1024], f32, tag="big", name="ps_v")
                    ps_va = pop.tile([128, 256], f32, tag="po", name="ps_va")
                    ps_vb = pop.tile([128, 256], f32, tag="po", name="ps_vb")
                    vdst = [
                        ps_va[:, 0:256],
                        ps_vb[:, 0:256],
                        ps_v[:, 0:256],
                        ps_v[:, 512:768],
                    ]
                    for c in range(CT):
                        if b == 0 and j == 0 and c < 4:
                            xt = xwarm[c]
                        else:
                            xt = xpool.tile([128, 512], bf16, tag="xt")
                            nc.gpsimd.dma_start(
                                out=xt[:, :],
                                in_=xT[c * 128 : (c + 1) * 128, n0 + j * 512 : n0 + (j + 1) * 512],
                            )
                        for part in range(4):  # QE, QO, KE, KO
                            dst = (ps_q, ps_q, ps_k, ps_k)[part]
                            off = (0, 512, 0, 512)[part]
                            wsl = wqk_sb[:, c * 512 + part * 128 : c * 512 + (part + 1) * 128]
                            nc.tensor.matmul(
                                dst[:, off : off + 512],
                                wsl,
                                xt[:, :],
                                start=(c == 0),
                                stop=(c == CT - 1),
                            )
                        for tl in range(4):
                            nc.tensor.matmul(
                                vdst[tl],
                                xt[:, tl * 128 : (tl + 1) * 128],
                                wv_sb[:, c * 256 : (c + 1) * 256],
                                start=(c == 0),
                                stop=(c == CT - 1),
                            )
                    # v copies first (frees the v psum slots for j+1)
                    for tl in range(4):
                        tt = j * 4 + tl
                        base = tt * VW
                        for h in range(HPC):
                            nc.vector.tensor_copy(
                                v_sb[:, base + h * 130 : base + h * 130 + 128],
                                vdst[tl][:, h * 128 : (h + 1) * 128],
                            )
                    # ACT copy frees psum banks fast, quantizing to bf16 so
                    # rope runs at DVE 2x; rope of the LAST j block is
                    # deferred off the attention-start critical path.
                    pcs = []
                    for qk in range(2):  # 0 = q, 1 = k
                        pc = tmppool.tile([128, 1024], bf16, tag="rt", name=f"pc{qk}")
                        nc.scalar.copy(pc[:, :], (ps_q, ps_k)[qk][:, :])
                        pcs.append(pc)

                    def emit_rope(j=j, js=js, pcs=pcs):
                        ce = cos_sb[:, n0 + j * 512 : n0 + (j + 1) * 512]
                        se = sin_sb[:, n0 + j * 512 : n0 + (j + 1) * 512]
                        for qk in range(2):
                            E_sb, O_sb = pcs[qk][:, 0:512], pcs[qk][:, 512:1024]
                            dst_e, dst_o = rot[qk]
                            t1 = tmppool.tile([128, 512], bf16, tag="rt2")
                            t2 = tmppool.tile([128, 512], bf16, tag="rt2")
                            nc.vector.tensor_mul(t1[:, :], E_sb, ce)
                            nc.vector.tensor_mul(t2[:, :], O_sb, se)
                            nc.vector.tensor_sub(dst_e[:, js], t1[:, :], t2[:, :])
                            t3 = tmppool.tile([128, 512], bf16, tag="rt2")
                            t4 = tmppool.tile([128, 512], bf16, tag="rt2")
                            nc.vector.tensor_mul(t3[:, :], E_sb, se)
                            nc.vector.tensor_mul(t4[:, :], O_sb, ce)
                            nc.vector.tensor_add(dst_o[:, js], t3[:, :], t4[:, :])
                        for h in range(HPC):
                            hb = 64 * h
                            nc.sync.dma_start(out=qhat[h][0:64, js], in_=qe2[hb : hb + 64, js])
                            nc.sync.dma_start(out=qhat[h][64:128, js], in_=qo2[hb : hb + 64, js])
                            nc.sync.dma_start(out=khat[h][0:64, js], in_=ke2[hb : hb + 64, js])
                            nc.sync.dma_start(out=khat[h][64:128, js], in_=ko2[hb : hb + 64, js])

                    if j < NB - 1:
                        emit_rope()
                        drain(2)
                    else:
                        deferred_rope.append(emit_rope)

                # ---- attention per head: paired tq blocks (jlo, jhi) share
                # one [128,1024] score psum + one wide exp instruction ----
                yT = [eopool.tile([128, T], bf16, tag="yt", name=f"yT{_h}") for _h in range(HPC)]
                for jp in range(NB // 2):
                    jlo, jhi = 2 * jp, 2 * jp + 1
                    for h in range(HPC):
                        if jp == 0 and h == 1 and deferred_rope:
                            deferred_rope.pop(0)()
                        exp_of = {}  # i -> (tile, base col of jlo half or None)
                        for i in range(4 * jhi + 4):
                            isl = slice(i * 128, (i + 1) * 128)
                            combined = i <= 4 * jlo + 3
                            sc = pbig.tile([128, 1024], f32, tag="big", name="sc")
                            ex = exppool.tile([128, 1024], bf16, tag="ex")
                            if combined:
                                nc.tensor.matmul(
                                    sc[:, 0:512], khat[h][:, isl],
                                    qhat[h][:, jlo * 512 : (jlo + 1) * 512],
                                    start=True, stop=True,
                                )
                                nc.tensor.matmul(
                                    sc[:, 512:1024], khat[h][:, isl],
                                    qhat[h][:, jhi * 512 : (jhi + 1) * 512],
                                    start=True, stop=True,
                                )
                                nc.scalar.activation(
                                    ex[:, :], sc[:, :],
                                    mybir.ActivationFunctionType.Exp, scale=SCALE,
                                )
                                p = i - 4 * jlo
                                if p >= 0:
                                    nc.gpsimd.tensor_mul(
                                        ex[:, p * 128 : (p + 1) * 128],
                                        ex[:, p * 128 : (p + 1) * 128],
                                        mask_sb[:, :],
                                    )
                                exp_of[i] = (ex, 0)
                            else:
                                nc.tensor.matmul(
                                    sc[:, 0:512], khat[h][:, isl],
                                    qhat[h][:, jhi * 512 : (jhi + 1) * 512],
                                    start=True, stop=True,
                                )
                                nc.scalar.activation(
                                    ex[:, 0:512], sc[:, 0:512],
                                    mybir.ActivationFunctionType.Exp, scale=SCALE,
                                )
                                p = i - 4 * jhi
                                if p >= 0:
                                    nc.gpsimd.tensor_mul(
                                        ex[:, p * 128 : (p + 1) * 128],
                                        ex[:, p * 128 : (p + 1) * 128],
                                        mask_sb[:, :],
                                    )
                                exp_of[i] = (ex, None)
                            if i % 2 == 1:
                                drain(1, "vector")

                        for j in (jlo, jhi):
                            half = 0 if j == jlo else 512
                            # 2 taus per packed psum tile (2x129 f32 regions)
                            ypk = [pbig.tile([128, 1024], f32, tag="big", name=f"ypk{_p}") for _p in range(2)]
                            for tau in range(4):
                                g = 4 * j + tau
                                dst = ypk[tau // 2]
                                yof = (tau % 2) * 512
                                for i in range(g + 1):
                                    ex, base = exp_of[i]
                                    col = (half if base == 0 else 0) + tau * 128
                                    nc.tensor.matmul(
                                        dst[:, yof : yof + 129],
                                        ex[:, col : col + 128],
                                        v_sb[:, i * VW + h * 130 : i * VW + h * 130 + 129],
                                        start=(i == 0),
                                        stop=(i == g),
                                    )
                            y_sb4 = []
                            for tau in range(4):
                                srcp = ypk[tau // 2]
                                yof = (tau % 2) * 512
                                r = rpool.tile([128, 1], f32, tag="r")
                                nc.vector.reciprocal(r[:, :], srcp[:, yof + 128 : yof + 129])
                                y_sb = ypool.tile([128, 128], bf16, tag="y")
                                nc.vector.tensor_scalar_mul(
                                    y_sb[:, :], srcp[:, yof : yof + 128], r[:, 0:1]
                                )
                                y_sb4.append(y_sb)

                            def finisher(j=j, h=h, y_sb4=y_sb4, n0=n0, yT=yT):
                                for tau in range(4):
                                    g = 4 * j + tau
                                    yt_ps = pop.tile([128, 128], bf16, tag="po", name="yt_ps")
                                    nc.tensor.transpose(yt_ps[:, :], y_sb4[tau][:, :], ident[:, :])
                                    nc.vector.tensor_copy(
                                        yT[h][:, g * 128 : (g + 1) * 128], yt_ps[:, :]
                                    )
                                if h == HPC - 1:
                                    push_wo(n0, yT, range(4 * j, 4 * j + 4))
                            finq.append(finisher)
                while finq:
                    finq.pop(0)()
            while finq or wo_jobs:
                drain(1, "mixed")

    nc.finalize()
    return nc


def _prep_inputs(x, w_qkv, w_o, rope_cos, rope_sin):
    import ml_dtypes

    bf = ml_dtypes.bfloat16
    xTh = np.ascontiguousarray(x.reshape(N, C).T).astype(bf)
    cosT = np.ascontiguousarray(rope_cos.T)  # [64, T]
    sinT = np.ascontiguousarray(rope_sin.T)
    cos2 = np.tile(np.concatenate([cosT, cosT], 0), (1, B)).astype(bf)
    sin2 = np.tile(np.concatenate([sinT, sinT], 0), (1, B)).astype(bf)

    r = np.arange(128)[:, None]
    c = np.arange(128)[None, :]
    mk = ((c - r) >= 0).astype(np.float32).astype(bf)

    ev = np.arange(0, D, 2)
    od = np.arange(1, D, 2)
    in_maps = []
    for m in range(NCORES):
        h0, h1 = 2 * m, 2 * m + 1
        # blocks QE|QO|KE|KO; within each, cols = [head0 dims | head1 dims]
        QE = np.concatenate([w_qkv[h0 * D + ev, :], w_qkv[h1 * D + ev, :]], 0).T
        QO = np.concatenate([w_qkv[h0 * D + od, :], w_qkv[h1 * D + od, :]], 0).T
        KE = np.concatenate([w_qkv[C + h0 * D + ev, :], w_qkv[C + h1 * D + ev, :]], 0).T
        KO = np.concatenate([w_qkv[C + h0 * D + od, :], w_qkv[C + h1 * D + od, :]], 0).T
        wqk_m = np.ascontiguousarray(np.concatenate([QE, QO, KE, KO], 1)).astype(bf)
        wv_m = np.ascontiguousarray(
            w_qkv[2 * C + 2 * m * D : 2 * C + (2 * m + 2) * D, :].T
        ).astype(bf)
        wo_m = np.ascontiguousarray(w_o[:, 2 * m * D : (2 * m + 2) * D].T).astype(bf)
        in_maps.append(
            {
                "xT": xTh,
                "w_qk": wqk_m,
                "w_v": wv_m,
                "w_o": wo_m,
                "cos2": cos2,
                "sin2": sin2,
                "masks": np.ascontiguousarray(mk),
            }
        )
    return in_maps


def kernel(x, w_qkv, w_o, rope_cos, rope_sin, _trace=False):
    global _COMPILED
    x = np.asarray(x, dtype=np.float32)
    w_qkv = np.asarray(w_qkv, dtype=np.float32)
    w_o = np.asarray(w_o, dtype=np.float32)
    rope_cos = np.asarray(rope_cos, dtype=np.float32)
    rope_sin = np.asarray(rope_sin, dtype=np.float32)

    from concourse.bass_utils import run_bass_kernel_spmd

    if _COMPILED is None:
        _COMPILED = _build()
    nc = _COMPILED
    in_maps = _prep_inputs(x, w_qkv, w_o, rope_cos, rope_sin)
    res = run_bass_kernel_spmd(
        nc, in_maps, core_ids=list(range(NCORES)), trace=_trace
    )
    out = np.zeros((N, C), dtype=np.float32)
    for m in range(NCORES):
        out += np.asarray(res.results[m]["out_p"], dtype=np.float32)
    kernel._last_results = res
    return out.reshape(B, T, C)
